# revision 40
# baseline (speedup 1.0000x reference)
"""Capsule-routing kernel for 8 TRN2 NeuronCores.

Strategy (n-sharded, u_hat never materialized):
  u_hat[b,n,c,d] = sum_i u[b,n,i] W[n,c,i,d] is only ever needed inside two
  contractions per routing iteration, both of which factor through W:
    (A) logits[b,n,c] = sum_d u_hat . Vacc  = sum_i u[b,n,i] * WV[b,n,c,i]
        with WV[b,n,c,i] = sum_d W[n,c,i,d] Vacc[b,c,d]   (PE matmul, p=d,
        bf16, out packed [(g4,b32); nl,i] for full-width DVE work)
    (B) s[b,c,d] = sum_n coup . u_hat = sum_{n,i} (coup[b,n,c] u[b,n,i]) W[n,c,i,d]
        p = (n16,i8) packed chunks, m = (c,d) groups of 8+2 capsules,
        f = (c-in-group, b) SLICED to the group's capsules so only the
        diagonal c blocks are streamed (f = 256 / 64).
  coup lives as [nl128; c,g,b] after a PE transpose; a constant replication
  matmul (R_w) re-packs it to [(n16,i); c,b] per 16-capsule chunk for (B).
  Each core owns 512 of the 4096 input capsules. Per-round global sums are
  AllGathered (bf16, 10KB) and reduced on-chip; final reduce + squash on
  host. A tiny warm-up AllGather fires at t=0 so the cross-core rendezvous
  barrier overlaps the const DMA loads + round-0 compute.

Layouts (host-prepared; partition dim first):
  WA [128=(g4,d32) ; c10, nl128, i8]  d zero-padded 16->32, bf16
  WP [128=(n16,i8) ; ch32, c10, d16]  packed W, chunk = 16 capsules, bf16
  UP [128=(n16,i8) ; ch32, b32]       bf16
  u4 [128=(g4,b32) ; nl128, i8]       bf16
  RW [128=nl       ; w8, m128]  R_w[nl, (n16,i)] = (nl == w*16+n16)
  isid [128=(k8,d16); d16]      identity stack for the 8-way gathered reduce
  rep16 [16=d      ; g4, d32]   replication matrix for vrep update
"""

import sys
import ml_dtypes
import numpy as np

sys.path.insert(0, "/opt/trn_rl_repo")

from contextlib import ExitStack

import concourse.bass as bass
import concourse.tile as tile
from concourse import bacc, mybir, masks
from concourse.bass_utils import run_bass_kernel_spmd

F32 = mybir.dt.float32
BF16 = mybir.dt.bfloat16
AX = mybir.AxisListType
ALU = mybir.AluOpType
ACTF = mybir.ActivationFunctionType

B, N, C, DI, DV = 32, 4096, 10, 8, 16
NCORES = 8
NL = N // NCORES          # 512 capsules per core
G = 4                     # n-groups per core
NLG = NL // G             # 128 capsules per group
NCH = NL * DI // 128      # 32 packed (n16,i) chunks
NUM_ROUTING = 3
EPS = 1e-7


def _body(ctx, tc, dins, out_d):
    nc = tc.nc

    consts = ctx.enter_context(tc.tile_pool(name="consts", bufs=1))
    persist = ctx.enter_context(tc.tile_pool(name="persist", bufs=1))
    work = ctx.enter_context(tc.tile_pool(name="work", bufs=4))
    psum_big = ctx.enter_context(tc.tile_pool(name="psum_big", bufs=2, space="PSUM"))
    psum_sm = ctx.enter_context(tc.tile_pool(name="psum_sm", bufs=3, space="PSUM"))
    dram = ctx.enter_context(tc.tile_pool(name="dram", bufs=1, space="DRAM"))

    # ---- constant / input loads ------------------------------------------
    wa = consts.tile([128, C, NLG, DI], BF16)
    wp_b = consts.tile([128, NCH, C, DV], BF16)
    up_b = consts.tile([128, NCH, B], BF16)
    u4 = consts.tile([G * B, NLG, DI], BF16)
    rw = consts.tile([NLG, 8, 128], BF16)
    isid = consts.tile([128, DV], BF16)
    gmask = consts.tile([128, G, B], BF16)
    rep16 = consts.tile([DV, G * 32], BF16)
    ident = consts.tile([128, 128], BF16)
    ones16 = consts.tile([DV, DV], BF16)

    # round-0-critical loads on the sync queue first
    nc.sync.dma_start(wp_b[:], dins["wp_b"].ap().rearrange(
        "p (ch c d) -> p ch c d", ch=NCH, c=C))
    nc.sync.dma_start(up_b[:], dins["up_b"].ap().rearrange(
        "p (ch b) -> p ch b", ch=NCH))
    # later-needed loads triggered from the (idle) ACT / DVE queues
    nc.scalar.dma_start(wa[:], dins["wa"].ap().rearrange(
        "p (c nl i) -> p c nl i", c=C, nl=NLG))
    nc.scalar.dma_start(u4[:], dins["u4"].ap().rearrange(
        "p (nl i) -> p nl i", nl=NLG))
    nc.scalar.dma_start(rw[:], dins["rw"].ap().rearrange(
        "p (w m) -> p w m", w=8))
    nc.scalar.dma_start(isid[:], dins["isid"].ap())
    nc.scalar.dma_start(gmask[:], dins["gmask"].ap().rearrange(
        "p (g b) -> p g b", g=G))
    nc.scalar.dma_start(rep16[:], dins["rep16"].ap())
    masks.make_identity(nc, ident[:])
    nc.gpsimd.memset(ones16[:], 1.0)

    # constant APs for activation bias operands
    czero = consts.tile([128, 1], F32)
    nc.gpsimd.memset(czero[:], 0.0)
    nc.const_aps.aps[(F32, 0.0)] = czero[:]
    ceps = consts.tile([128, 1], F32)
    nc.gpsimd.memset(ceps[:], EPS)
    nc.const_aps.aps[(F32, EPS)] = ceps[:]

    # persistent accumulator: block-diagonal bf16 Vacc for (A):
    # blk[(g,d32),c,(g',b)] = Vacc[b,c,d]*delta_gg'. Accumulated directly
    # (blk += srep*gmask each round), split lo(c<4)/hi(c>=4) so the (A)
    # c=0 matmul unblocks after the small lo update instead of the full one.
    blk_lo = persist.tile([128, 4, G, B], BF16)
    blk_hi = persist.tile([128, C - 4, G, B], BF16)
    nc.gpsimd.memset(blk_lo[:], 0.0)
    nc.gpsimd.memset(blk_hi[:], 0.0)

    def blk_for(c):
        return blk_lo[:, c] if c < 4 else blk_hi[:, c - 4]

    coup_t = persist.tile([NLG, C, G, B], BF16)  # [nl; c, g, b]
    cup = persist.tile([128, NCH, C, B], BF16)   # [(n16,i); ch, c, b] packed cu
    logits = persist.tile([G * B, C, NLG], BF16)

    def squash_update(ps_tot):
        """squash scale from stot in PSUM [16; (c,b)], fold into blk.

        scale = s2/(1+s2)/sqrt(s2+eps); one approx reciprocal of the
        combined denominator (1+s2)*sqrt(s2+eps) replaces the two exact
        reciprocals."""
        sq = work.tile([DV, C * B], BF16, tag="sq")
        nc.scalar.square(sq[:], ps_tot[:])
        ps_s2 = psum_sm.tile([DV, C * B], F32, tag="ps_misc")
        nc.tensor.matmul(ps_s2[:], lhsT=ones16[:], rhs=sq[:], start=True,
                         stop=True)
        rt = work.tile([DV, C * B], F32, tag="rt")
        nc.scalar.activation(rt[:], ps_s2[:], ACTF.Sqrt, bias=EPS)
        dn = work.tile([DV, C * B], F32, tag="dn")
        nc.vector.scalar_tensor_tensor(
            out=dn[:], in0=ps_s2[:], scalar=1.0, in1=rt[:],
            op0=ALU.add, op1=ALU.mult)
        rr = work.tile([DV, C * B], F32, tag="rr")
        nc.vector.reciprocal_approx_fast(rr[:], dn[:])
        sc = work.tile([DV, C * B], F32, tag="sc")
        nc.vector.tensor_tensor(out=sc[:], in0=ps_s2[:], in1=rr[:],
                                op=ALU.mult)
        vsmall = work.tile([DV, C * B], BF16, tag="vsmall")
        nc.vector.tensor_tensor(out=vsmall[:], in0=ps_tot[:], in1=sc[:],
                                op=ALU.mult)
        ps_srep = psum_sm.tile([128, C * B], F32, tag="ps_misc")
        nc.tensor.matmul(ps_srep[:], lhsT=rep16[:], rhs=vsmall[:],
                         start=True, stop=True)
        srep = ps_srep[:].rearrange("p (c b) -> p c b", c=C)
        for blk, c0, c1 in ((blk_lo, 0, 4), (blk_hi, 4, C)):
            ncs = c1 - c0
            tmp = work.tile([128, ncs, G, B], BF16, tag=f"vtmp{c0}")
            nc.vector.tensor_tensor(
                out=tmp[:],
                in0=srep[:, c0:c1].unsqueeze(2).broadcast_to(
                    (128, ncs, G, B)),
                in1=gmask[:].unsqueeze(1).broadcast_to((128, ncs, G, B)),
                op=ALU.mult)
            nc.vector.tensor_tensor(out=blk[:], in0=blk[:], in1=tmp[:],
                                    op=ALU.add)

    def gather(write_part, rnd):
        """AllGather the bf16 partial sum in [d; (c,b)] DRAM layout, reduce
        the 8 gathered copies with one PE matmul -> stot PSUM [16; (c,b)]."""
        d_part = dram.tile([DV, C * B], BF16, tag=f"dpart{rnd}")
        d_gath = dram.tile([NCORES * DV, C * B], BF16, tag=f"dgath{rnd}")
        write_part(d_part)
        nc.gpsimd.collective_compute(
            "AllGather", ALU.bypass, replica_groups=[list(range(NCORES))],
            ins=[d_part[:].opt()], outs=[d_gath[:].opt()])
        gath = work.tile([128, C * B], BF16, tag="gath")
        nc.sync.dma_start(gath[:, 0:160], d_gath[:, 0:160])
        nc.scalar.dma_start(gath[:, 160:320], d_gath[:, 160:320])
        ps_tot = psum_sm.tile([DV, C * B], F32, tag="ps_misc")
        nc.tensor.matmul(ps_tot[:], lhsT=isid[:], rhs=gath[:], start=True,
                         stop=True)
        return ps_tot

    GROUPS = ((0, 8), (8, 2))   # (c_base, n_capsules) -> m = 128 / 32

    def bsum(rhs_for):
        """(B): psum groups [(c,d16); f], PSUM-accumulated over the 32
        packed chunks. Group 0 = capsules 0-7, group 1 = 8-9. rhs_for
        returns the group-sliced moving tensor, so only the diagonal c
        blocks are streamed."""
        groups = []
        for c0, nc_ in GROUPS:
            fdim = rhs_for(0, c0, nc_).free_size()
            ph = psum_big.tile([16 * nc_, fdim], F32, tag=f"ps_b{c0}", bufs=1)
            for ch in range(NCH):
                nc.tensor.matmul(
                    ph[:],
                    lhsT=wp_b[:, ch, c0:c0 + nc_, :].rearrange(
                        "p c d -> p (c d)"),
                    rhs=rhs_for(ch, c0, nc_),
                    start=(ch == 0), stop=(ch == NCH - 1),
                    skip_group_check=True,
                )
            groups.append(ph)
        return groups

    def diag_out(groups, dst_for):
        """Copy psum groups to SBUF (bf16), then DMA the diagonal [16;32]
        blocks to DRAM, alternating trigger queues to halve the serial
        dma_start cost. dst_for(c) gives the [16, 32] dest AP."""
        qi = 0
        for (c0, nc_), grp in reversed(list(zip(GROUPS, groups))):
            s_f = work.tile([16 * nc_, nc_ * B], BF16, tag=f"s_fd{c0}")
            nc.scalar.copy(s_f[:], grp[:])
            for cl in range(nc_):
                eng = nc.sync if qi % 2 == 0 else nc.scalar
                qi += 1
                eng.dma_start(
                    dst_for(c0 + cl),
                    s_f[16 * cl:16 * (cl + 1), cl * B:(cl + 1) * B])

    def round0():
        groups = bsum(lambda ch, c0, nc_: up_b[:, ch, :])
        s_h = []
        for (c0, nc_), grp in zip(GROUPS, groups):
            t = work.tile([16 * nc_, B], BF16, tag=f"s_h{c0}")
            nc.scalar.activation(t[:], grp[:], ACTF.Copy, scale=1.0 / C)
            s_h.append(t)

        def write_part(d_part):
            dp = d_part[:].rearrange("d (c b) -> d c b", c=C)
            for (c0, nc_), t in zip(GROUPS, s_h):
                nc.sync.dma_start(
                    dp[:, c0:c0 + nc_, :].transpose([1, 0, 2]),
                    t[:])
        return gather(write_part, 0)

    def round12(rnd, last):
        """One routing round, software-pipelined over the two nl-halves:
        while DVE/ACT run half-0's softmax->repack chain, PE runs half-1's
        (A) matmuls, so neither engine stalls on the other's serial chain.
        (B) psum groups accumulate across both halves' chunks."""
        HNL = NLG // 2            # 64 capsules per (g, half)
        bgroups = []
        for c0, nc_ in GROUPS:
            ph = psum_big.tile([16 * nc_, nc_ * B], F32,
                               tag=f"ps_b{c0}", bufs=1)
            bgroups.append(ph)

        for h in range(2):
            # ---- (A): WV then logits for this half -----------------------
            lg = work.tile([G * B, C, HNL], BF16, tag="lg")
            with nc.allow_low_precision(
                    reason="bf16 logits within 2e-2 budget"):
                for c in range(C):
                    ps_wv = psum_big.tile([128, HNL, DI], F32, tag="ps_wv",
                                          bufs=3)
                    nc.tensor.matmul(
                        ps_wv[:].rearrange("p nl i -> p (nl i)"),
                        lhsT=blk_for(c).rearrange("p g b -> p (g b)"),
                        rhs=wa[:, c, 64 * h:64 * (h + 1), :],
                        start=True, stop=True,
                    )
                    wv_sb = work.tile([128, HNL, DI], BF16, tag="wv_sb")
                    nc.scalar.copy(wv_sb[:], ps_wv[:])
                    wvu = work.tile([128, HNL, DI], BF16, tag="wvu")
                    nc.vector.tensor_tensor(out=wvu[:], in0=wv_sb[:],
                                            in1=u4[:, 64 * h:64 * (h + 1), :],
                                            op=ALU.mult)
                    nc.vector.tensor_reduce(
                        lg[:, c, :], wvu[:], axis=AX.X, op=ALU.add)

            # ---- softmax over c for this half ----------------------------
            expd = work.tile([G * B, C, HNL], BF16, tag="expd")
            nc.scalar.activation(expd[:], lg[:], ACTF.Exp)
            den = work.tile([G * B, HNL], F32, tag="den")
            nc.vector.tensor_reduce(
                den[:], expd[:].transpose([0, 2, 1]), axis=AX.X, op=ALU.add)
            rden = work.tile([G * B, HNL], F32, tag="rden")
            nc.vector.reciprocal_approx_fast(rden[:], den[:])
            coupq = work.tile([G * B, C, HNL], BF16, tag="coupq")
            nc.vector.tensor_tensor(
                out=coupq[:], in0=expd[:],
                in1=rden[:].unsqueeze(1).broadcast_to((G * B, C, HNL)),
                op=ALU.mult)

            # ---- transpose coup to [nl; c,g,b] rows of this half ---------
            # two capsules per transpose (m = 2*64 = 128 stationary rows)
            for c in range(0, C, 2):
                ps_tr = psum_sm.tile([2 * HNL, G * B], BF16, tag="ps_misc")
                nc.tensor.transpose(
                    ps_tr[:],
                    coupq[:, c:c + 2, :].rearrange("p c nl -> p (c nl)"),
                    ident[:])
                for cl in range(2):
                    nc.scalar.copy(
                        coup_t[64 * h:64 * (h + 1), c + cl, :, :].rearrange(
                            "p g b -> p (g b)"),
                        ps_tr[cl * HNL:(cl + 1) * HNL, :])

            # ---- repack + fold u for this half's 16 chunks ---------------
            for wl in range(4):
                w = 4 * h + wl
                for g in range(G):
                    ch = g * 8 + w
                    ps_rep = psum_sm.tile([128, C, B], F32, tag="ps_misc")
                    nc.tensor.matmul(
                        ps_rep[:].rearrange("p c b -> p (c b)"),
                        lhsT=rw[64 * h:64 * (h + 1), w, :],
                        rhs=coup_t[64 * h:64 * (h + 1), :, g, :],
                        start=True, stop=True,
                    )
                    nc.vector.tensor_tensor(
                        out=cup[:, ch, :, :],
                        in0=ps_rep[:],
                        in1=up_b[:, ch, :].unsqueeze(1).broadcast_to(
                            (128, C, B)),
                        op=ALU.mult)
            # ---- (B) accumulate this half's chunks (emitted after the
            # repacks so the PE doesn't stall per chunk on the cup mults).
            # In the final half emit group1 before group0 per chunk so the
            # small group's psum closes first and its output copy overlaps
            # group0's last matmuls.
            grp_order = list(zip(GROUPS, bgroups))
            if h == 1:
                grp_order = grp_order[::-1]
            for wl in range(4):
                w = 4 * h + wl
                for g in range(G):
                    ch = g * 8 + w
                    for (c0, nc_), ph in grp_order:
                        nc.tensor.matmul(
                            ph[:],
                            lhsT=wp_b[:, ch, c0:c0 + nc_, :].rearrange(
                                "p c d -> p (c d)"),
                            rhs=cup[:, ch, c0:c0 + nc_, :],
                            start=(h == 0 and wl == 0 and g == 0),
                            stop=(h == 1 and wl == 3 and g == G - 1),
                            skip_group_check=True,
                        )
        groups = bgroups

        if last:
            # final round: dump the two psum groups whole (2 DMAs instead of
            # 10 tiny diagonal-block DMAs); host extracts the diagonals
            for (c0, nc_), grp in reversed(list(zip(GROUPS, groups))):
                s_f = work.tile([16 * nc_, nc_ * B], BF16, tag=f"s_f{c0}")
                nc.scalar.copy(s_f[:], grp[:])
                if c0 == 0:
                    # split across two trigger queues / DMA engines
                    nc.sync.dma_start(out_d.ap()[:, 0:128], s_f[:, 0:128])
                    nc.scalar.dma_start(out_d.ap()[:, 128:256],
                                        s_f[:, 128:256])
                else:
                    nc.sync.dma_start(
                        out_d.ap()[0:16 * nc_, 256:256 + nc_ * B], s_f[:])
            return None

        def write_part(d_part):
            dp = d_part[:].rearrange("d (c b) -> d c b", c=C)
            diag_out(groups, lambda c: dp[:, c, :])
        return gather(write_part, rnd)

    stot = round0()
    squash_update(stot)
    stot = round12(1, last=False)
    squash_update(stot)
    round12(2, last=True)


IN_SHAPES = {
    "wa": [128, C * NLG * DI],
    "u4": [G * B, NLG * DI],
    "rw": [NLG, 8 * 128],
    "wp_b": [128, NCH * C * DV],
    "up_b": [128, NCH * B],
    "isid": [128, DV],
    "rep16": [DV, G * 32],
    "gmask": [128, G * B],
}


def build_nc():
    nc = bacc.Bacc("TRN2", target_bir_lowering=False, debug=False,
                   num_devices=NCORES)
    dins = {name: nc.dram_tensor(name, shape, BF16, kind="ExternalInput")
            for name, shape in IN_SHAPES.items()}
    # out: group0 (capsules 0-7) full [128,(8c,b)] dump in cols 0:256,
    # group1 (capsules 8-9) [32,(2c,b)] in cols 256:320 rows 0:32
    out_d = nc.dram_tensor("out", [128, 320], BF16, kind="ExternalOutput")

    with tile.TileContext(nc) as tc, ExitStack() as ctx:
        _body(ctx, tc, dins, out_d)
    nc.compile()
    return nc


# --------------------------------------------------------------------------
# Host side
# --------------------------------------------------------------------------

def make_in_maps(x, W):
    x = np.ascontiguousarray(np.asarray(x, dtype=np.float32))
    W = np.ascontiguousarray(np.asarray(W, dtype=np.float32))
    u = x.reshape(B, N, DI)
    isid = np.tile(np.eye(DV, dtype=np.float32), (NCORES, 1)).astype(
        ml_dtypes.bfloat16)
    gmask = np.ascontiguousarray(
        np.kron(np.eye(G, dtype=np.float32), np.ones((32, 1), np.float32))
        .reshape(128, G, 1) * np.ones((1, 1, B), np.float32)
        ).reshape(128, G * B).astype(ml_dtypes.bfloat16)
    rep16 = np.zeros((DV, G, 32), np.float32)
    for d in range(DV):
        rep16[d, :, d] = 1.0
    rep16 = rep16.reshape(DV, G * 32).astype(ml_dtypes.bfloat16)
    rwm = np.zeros((NLG, 8, 128), np.float32)
    for w in range(8):
        for n16 in range(16):
            rwm[w * 16 + n16, w, n16 * DI:(n16 + 1) * DI] = 1.0
    rwm = rwm.reshape(NLG, 8 * 128).astype(ml_dtypes.bfloat16)

    in_maps = []
    for k in range(NCORES):
        sl = u[:, k * NL:(k + 1) * NL, :]                   # [B, 512, 8]
        Wk = W[k * NL:(k + 1) * NL]                         # [512, C, DI, DV]
        Wk_g = Wk.reshape(G, NLG, C, DI, DV)
        wa = np.zeros((G, 32, C, NLG, DI), np.float32)
        wa[:, :DV] = Wk_g.transpose(0, 4, 2, 1, 3)          # [g,d,c,nl,i]
        # packed: p = (n16, i), chunks of 16 n
        Wp = Wk.reshape(NCH, 16, C, DI, DV).transpose(1, 3, 0, 2, 4)
        # -> [n16, i, ch, c, d]
        Up = sl.reshape(B, NCH, 16, DI).transpose(2, 3, 1, 0)  # [n16,i,ch,b]
        u4 = sl.reshape(B, G, NLG, DI).transpose(1, 0, 2, 3)   # [g,b,nl,i]
        in_maps.append({
            "wa": np.ascontiguousarray(
                wa.reshape(128, C * NLG * DI)).astype(ml_dtypes.bfloat16),
            "wp_b": np.ascontiguousarray(
                Wp.reshape(128, NCH * C * DV)).astype(ml_dtypes.bfloat16),
            "up_b": np.ascontiguousarray(
                Up.reshape(128, NCH * B)).astype(ml_dtypes.bfloat16),
            "u4": np.ascontiguousarray(
                u4.reshape(G * B, NLG * DI)).astype(ml_dtypes.bfloat16),
            "rw": rwm,
            "isid": isid,
            "rep16": rep16,
            "gmask": gmask,
        })
    return in_maps


def postprocess(outs):
    """outs: list (per core) of [128, 320] bf16 full psum-group dumps.
    Diagonal blocks: group0 s[c,d,b] = o[c*16+d, c*32+b] (c<8), group1
    s[8+c,d,b] = o[c*16+d, 256+c*32+b] (c<2). Sum cores, squash."""
    s = np.zeros((C, DV, B), np.float64)
    i8, i2 = np.arange(8), np.arange(2)
    for o in outs:
        o = np.asarray(o, np.float64)
        s[:8] += o[:, :256].reshape(8, DV, 8, B)[i8, :, i8, :]
        s[8:] += o[:32, 256:].reshape(2, DV, 2, B)[i2, :, i2, :]
    s = s.transpose(2, 0, 1)                                # [b, c, d]
    s2 = np.sum(s * s, axis=-1, keepdims=True)
    v = (s2 / (1.0 + s2) / np.sqrt(s2 + EPS)) * s
    return v.astype(np.float32)


_NC_CACHE = {}


def kernel(x, W):
    if "nc" not in _NC_CACHE:
        _NC_CACHE["nc"] = build_nc()
    nc = _NC_CACHE["nc"]
    in_maps = make_in_maps(x, W)
    res = run_bass_kernel_spmd(nc, in_maps, list(range(NCORES)))
    outs = [res.results[k]["out"] for k in range(NCORES)]
    return postprocess(outs)


# revision 42
# speedup vs baseline: 1.0887x; 1.0887x over previous
"""Capsule-routing kernel for 8 TRN2 NeuronCores.

Strategy (n-sharded, u_hat never materialized):
  u_hat[b,n,c,d] = sum_i u[b,n,i] W[n,c,i,d] is only ever needed inside two
  contractions per routing iteration, both of which factor through W:
    (A) logits[b,n,c] = sum_d u_hat . Vacc  = sum_i u[b,n,i] * WV[b,n,c,i]
        with WV[b,n,c,i] = sum_d W[n,c,i,d] Vacc[b,c,d]   (PE matmul, p=d,
        bf16, out packed [(g4,b32); nl,i] for full-width DVE work)
    (B) s[b,c,d] = sum_n coup . u_hat = sum_{n,i} (coup[b,n,c] u[b,n,i]) W[n,c,i,d]
        p = (n16,i8) packed chunks, m = (c,d) groups of 8+2 capsules,
        f = (c-in-group, b) SLICED to the group's capsules so only the
        diagonal c blocks are streamed (f = 256 / 64).
  coup lives as [nl128; c,g,b] after a PE transpose; a constant replication
  matmul (R_w) re-packs it to [(n16,i); c,b] per 16-capsule chunk for (B).
  Each core owns 512 of the 4096 input capsules. Per-round global sums are
  AllGathered (bf16, 10KB) and reduced on-chip; final reduce + squash on
  host. A tiny warm-up AllGather fires at t=0 so the cross-core rendezvous
  barrier overlaps the const DMA loads + round-0 compute.

Layouts (host-prepared; partition dim first):
  WA [128=(g4,d32) ; c10, nl128, i8]  d zero-padded 16->32, bf16
  WP [128=(n16,i8) ; ch32, c10, d16]  packed W, chunk = 16 capsules, bf16
  UP [128=(n16,i8) ; ch32, b32]       bf16
  u4 [128=(g4,b32) ; nl128, i8]       bf16
  RW [128=nl       ; w8, m128]  R_w[nl, (n16,i)] = (nl == w*16+n16)
  isid [128=(k8,d16); d16]      identity stack for the 8-way gathered reduce
  rep16 [16=d      ; g4, d32]   replication matrix for vrep update
"""

import sys
import ml_dtypes
import numpy as np

sys.path.insert(0, "/opt/trn_rl_repo")

from contextlib import ExitStack

import concourse.bass as bass
import concourse.tile as tile
from concourse import bacc, mybir, masks
from concourse.bass_utils import run_bass_kernel_spmd

F32 = mybir.dt.float32
BF16 = mybir.dt.bfloat16
AX = mybir.AxisListType
ALU = mybir.AluOpType
ACTF = mybir.ActivationFunctionType

B, N, C, DI, DV = 32, 4096, 10, 8, 16
NCORES = 8
NL = N // NCORES          # 512 capsules per core
G = 4                     # n-groups per core
NLG = NL // G             # 128 capsules per group
NCH = NL * DI // 128      # 32 packed (n16,i) chunks
NUM_ROUTING = 3
EPS = 1e-7


def _body(ctx, tc, dins, out_d):
    nc = tc.nc

    consts = ctx.enter_context(tc.tile_pool(name="consts", bufs=1))
    persist = ctx.enter_context(tc.tile_pool(name="persist", bufs=1))
    work = ctx.enter_context(tc.tile_pool(name="work", bufs=3))
    psum_big = ctx.enter_context(tc.tile_pool(name="psum_big", bufs=2, space="PSUM"))
    psum_sm = ctx.enter_context(tc.tile_pool(name="psum_sm", bufs=3, space="PSUM"))
    dram = ctx.enter_context(tc.tile_pool(name="dram", bufs=1, space="DRAM"))

    # ---- constant / input loads ------------------------------------------
    wa = consts.tile([128, C, NLG, DI], BF16)
    wp_b = consts.tile([128, NCH, C, DV], BF16)
    up_b = consts.tile([128, NCH, B], BF16)
    u4 = consts.tile([G * B, NLG, DI], BF16)
    rw = consts.tile([NLG, 8, 128], BF16)
    isid = consts.tile([128, DV], BF16)
    gmask = consts.tile([128, G, B], BF16)
    rep16 = consts.tile([DV, G * 32], BF16)
    ident = consts.tile([128, 128], BF16)
    ones16 = consts.tile([DV, DV], BF16)

    # round-0-critical loads on the sync queue first
    nc.sync.dma_start(wp_b[:], dins["wp_b"].ap().rearrange(
        "p (ch c d) -> p ch c d", ch=NCH, c=C))
    nc.sync.dma_start(up_b[:], dins["up_b"].ap().rearrange(
        "p (ch b) -> p ch b", ch=NCH))
    # later-needed loads triggered from the (idle) ACT / DVE queues
    nc.scalar.dma_start(wa[:], dins["wa"].ap().rearrange(
        "p (c nl i) -> p c nl i", c=C, nl=NLG))
    nc.scalar.dma_start(u4[:], dins["u4"].ap().rearrange(
        "p (nl i) -> p nl i", nl=NLG))
    nc.scalar.dma_start(rw[:], dins["rw"].ap().rearrange(
        "p (w m) -> p w m", w=8))
    nc.scalar.dma_start(isid[:], dins["isid"].ap())
    nc.scalar.dma_start(gmask[:], dins["gmask"].ap().rearrange(
        "p (g b) -> p g b", g=G))
    nc.scalar.dma_start(rep16[:], dins["rep16"].ap())
    masks.make_identity(nc, ident[:])
    nc.gpsimd.memset(ones16[:], 1.0)

    # constant APs for activation bias operands
    czero = consts.tile([128, 1], F32)
    nc.gpsimd.memset(czero[:], 0.0)
    nc.const_aps.aps[(F32, 0.0)] = czero[:]
    ceps = consts.tile([128, 1], F32)
    nc.gpsimd.memset(ceps[:], EPS)
    nc.const_aps.aps[(F32, EPS)] = ceps[:]

    # persistent accumulator: block-diagonal bf16 Vacc for (A):
    # blk[(g,d32),c,(g',b)] = Vacc[b,c,d]*delta_gg'. Accumulated directly
    # (blk += srep*gmask each round), split lo(c<4)/hi(c>=4) so the (A)
    # c=0 matmul unblocks after the small lo update instead of the full one.
    blk_lo = persist.tile([128, 4, G, B], BF16)
    blk_hi = persist.tile([128, C - 4, G, B], BF16)
    nc.gpsimd.memset(blk_lo[:], 0.0)
    nc.gpsimd.memset(blk_hi[:], 0.0)

    def blk_for(c):
        return blk_lo[:, c] if c < 4 else blk_hi[:, c - 4]

    coup_t = persist.tile([NLG, C, G, B], BF16)  # [nl; c, g, b]
    cup = persist.tile([128, NCH, C, B], BF16)   # [(n16,i); ch, c, b] packed cu
    logits = persist.tile([G * B, C, NLG], BF16)

    def squash_update(ps_tot):
        """squash scale from stot in PSUM [16; (c,b)], fold into blk.

        scale = s2/(1+s2)/sqrt(s2+eps); one approx reciprocal of the
        combined denominator (1+s2)*sqrt(s2+eps) replaces the two exact
        reciprocals."""
        sq = work.tile([DV, C * B], BF16, tag="sq")
        nc.scalar.square(sq[:], ps_tot[:])
        ps_s2 = psum_sm.tile([DV, C * B], F32, tag="ps_misc")
        nc.tensor.matmul(ps_s2[:], lhsT=ones16[:], rhs=sq[:], start=True,
                         stop=True)
        rt = work.tile([DV, C * B], F32, tag="rt")
        nc.scalar.activation(rt[:], ps_s2[:], ACTF.Sqrt, bias=EPS)
        dn = work.tile([DV, C * B], F32, tag="dn")
        nc.vector.scalar_tensor_tensor(
            out=dn[:], in0=ps_s2[:], scalar=1.0, in1=rt[:],
            op0=ALU.add, op1=ALU.mult)
        rr = work.tile([DV, C * B], F32, tag="rr")
        nc.vector.reciprocal_approx_fast(rr[:], dn[:])
        sc = work.tile([DV, C * B], F32, tag="sc")
        nc.vector.tensor_tensor(out=sc[:], in0=ps_s2[:], in1=rr[:],
                                op=ALU.mult)
        vsmall = work.tile([DV, C * B], BF16, tag="vsmall")
        nc.vector.tensor_tensor(out=vsmall[:], in0=ps_tot[:], in1=sc[:],
                                op=ALU.mult)
        ps_srep = psum_sm.tile([128, C * B], F32, tag="ps_misc")
        nc.tensor.matmul(ps_srep[:], lhsT=rep16[:], rhs=vsmall[:],
                         start=True, stop=True)
        srep = ps_srep[:].rearrange("p (c b) -> p c b", c=C)
        for blk, c0, c1 in ((blk_lo, 0, 4), (blk_hi, 4, C)):
            ncs = c1 - c0
            tmp = work.tile([128, ncs, G, B], BF16, tag=f"vtmp{c0}")
            nc.vector.tensor_tensor(
                out=tmp[:],
                in0=srep[:, c0:c1].unsqueeze(2).broadcast_to(
                    (128, ncs, G, B)),
                in1=gmask[:].unsqueeze(1).broadcast_to((128, ncs, G, B)),
                op=ALU.mult)
            nc.vector.tensor_tensor(out=blk[:], in0=blk[:], in1=tmp[:],
                                    op=ALU.add)

    def gather(write_part, rnd):
        """AllGather the bf16 partial sum in [d; (c,b)] DRAM layout, reduce
        the 8 gathered copies with one PE matmul -> stot PSUM [16; (c,b)]."""
        d_part = dram.tile([DV, C * B], BF16, tag=f"dpart{rnd}")
        d_gath = dram.tile([NCORES * DV, C * B], BF16, tag=f"dgath{rnd}")
        write_part(d_part)
        nc.gpsimd.collective_compute(
            "AllGather", ALU.bypass, replica_groups=[list(range(NCORES))],
            ins=[d_part[:].opt()], outs=[d_gath[:].opt()])
        gath = work.tile([128, C * B], BF16, tag="gath")
        nc.sync.dma_start(gath[:, 0:160], d_gath[:, 0:160])
        nc.scalar.dma_start(gath[:, 160:320], d_gath[:, 160:320])
        ps_tot = psum_sm.tile([DV, C * B], F32, tag="ps_misc")
        nc.tensor.matmul(ps_tot[:], lhsT=isid[:], rhs=gath[:], start=True,
                         stop=True)
        return ps_tot

    GROUPS = ((0, 8), (8, 2))   # (c_base, n_capsules) -> m = 128 / 32

    def bsum(rhs_for):
        """(B): psum groups [(c,d16); f], PSUM-accumulated over the 32
        packed chunks. Group 0 = capsules 0-7, group 1 = 8-9. rhs_for
        returns the group-sliced moving tensor, so only the diagonal c
        blocks are streamed."""
        groups = []
        for c0, nc_ in GROUPS:
            fdim = rhs_for(0, c0, nc_).free_size()
            ph = psum_big.tile([16 * nc_, fdim], F32, tag=f"ps_b{c0}", bufs=1)
            for ch in range(NCH):
                nc.tensor.matmul(
                    ph[:],
                    lhsT=wp_b[:, ch, c0:c0 + nc_, :].rearrange(
                        "p c d -> p (c d)"),
                    rhs=rhs_for(ch, c0, nc_),
                    start=(ch == 0), stop=(ch == NCH - 1),
                    skip_group_check=True,
                )
            groups.append(ph)
        return groups

    def diag_out(groups, dst_for):
        """Copy psum groups to SBUF (bf16), then DMA the diagonal [16;32]
        blocks to DRAM, alternating trigger queues to halve the serial
        dma_start cost. dst_for(c) gives the [16, 32] dest AP."""
        qi = 0
        for (c0, nc_), grp in zip(GROUPS, groups):
            s_f = work.tile([16 * nc_, nc_ * B], BF16, tag=f"s_fd{c0}")
            nc.scalar.copy(s_f[:], grp[:])
            for cl in range(nc_):
                eng = nc.sync if qi % 2 == 0 else nc.scalar
                qi += 1
                eng.dma_start(
                    dst_for(c0 + cl),
                    s_f[16 * cl:16 * (cl + 1), cl * B:(cl + 1) * B])

    def round0():
        groups = bsum(lambda ch, c0, nc_: up_b[:, ch, :])
        s_h = []
        for (c0, nc_), grp in zip(GROUPS, groups):
            t = work.tile([16 * nc_, B], BF16, tag=f"s_h{c0}")
            nc.scalar.activation(t[:], grp[:], ACTF.Copy, scale=1.0 / C)
            s_h.append(t)

        def write_part(d_part):
            dp = d_part[:].rearrange("d (c b) -> d c b", c=C)
            for (c0, nc_), t in zip(GROUPS, s_h):
                nc.sync.dma_start(
                    dp[:, c0:c0 + nc_, :].transpose([1, 0, 2]),
                    t[:])
        return gather(write_part, 0)

    def round12(rnd, last):
        """One routing round, software-pipelined over the two nl-halves:
        while DVE/ACT run half-0's softmax->repack chain, PE runs half-1's
        (A) matmuls, so neither engine stalls on the other's serial chain.
        (B) psum groups accumulate across both halves' chunks."""
        HNL = NLG // 2            # 64 capsules per (g, half)
        bgroups = []
        for c0, nc_ in GROUPS:
            ph = psum_big.tile([16 * nc_, nc_ * B], F32,
                               tag=f"ps_b{c0}", bufs=1)
            bgroups.append(ph)

        for h in range(2):
            # ---- (A): WV then logits for this half -----------------------
            lg = work.tile([G * B, C, HNL], BF16, tag="lg")
            with nc.allow_low_precision(
                    reason="bf16 logits within 2e-2 budget"):
                for c in range(C):
                    ps_wv = psum_big.tile([128, HNL, DI], F32, tag="ps_wv",
                                          bufs=3)
                    nc.tensor.matmul(
                        ps_wv[:].rearrange("p nl i -> p (nl i)"),
                        lhsT=blk_for(c).rearrange("p g b -> p (g b)"),
                        rhs=wa[:, c, 64 * h:64 * (h + 1), :],
                        start=True, stop=True,
                    )
                    wv_sb = work.tile([128, HNL, DI], BF16, tag="wv_sb")
                    nc.scalar.copy(wv_sb[:], ps_wv[:])
                    wvu = work.tile([128, HNL, DI], BF16, tag="wvu")
                    nc.vector.tensor_tensor(out=wvu[:], in0=wv_sb[:],
                                            in1=u4[:, 64 * h:64 * (h + 1), :],
                                            op=ALU.mult)
                    nc.vector.tensor_reduce(
                        lg[:, c, :], wvu[:], axis=AX.X, op=ALU.add)

            # ---- softmax over c for this half ----------------------------
            expd = work.tile([G * B, C, HNL], BF16, tag="expd")
            nc.scalar.activation(expd[:], lg[:], ACTF.Exp)
            den = work.tile([G * B, HNL], F32, tag="den")
            nc.vector.tensor_reduce(
                den[:], expd[:].transpose([0, 2, 1]), axis=AX.X, op=ALU.add)
            rden = work.tile([G * B, HNL], F32, tag="rden")
            nc.vector.reciprocal_approx_fast(rden[:], den[:])
            coupq = work.tile([G * B, C, HNL], BF16, tag="coupq")
            nc.vector.tensor_tensor(
                out=coupq[:], in0=expd[:],
                in1=rden[:].unsqueeze(1).broadcast_to((G * B, C, HNL)),
                op=ALU.mult)

            # ---- transpose coup to [nl; c,g,b] rows of this half ---------
            # two capsules per transpose (m = 2*64 = 128 stationary rows)
            for c in range(0, C, 2):
                ps_tr = psum_sm.tile([2 * HNL, G * B], BF16, tag="ps_misc")
                nc.tensor.transpose(
                    ps_tr[:],
                    coupq[:, c:c + 2, :].rearrange("p c nl -> p (c nl)"),
                    ident[:])
                for cl in range(2):
                    nc.scalar.copy(
                        coup_t[64 * h:64 * (h + 1), c + cl, :, :].rearrange(
                            "p g b -> p (g b)"),
                        ps_tr[cl * HNL:(cl + 1) * HNL, :])

            # ---- repack + fold u for this half's 16 chunks ---------------
            for wl in range(4):
                w = 4 * h + wl
                for g in range(G):
                    ch = g * 8 + w
                    ps_rep = psum_sm.tile([128, C, B], F32, tag="ps_misc")
                    nc.tensor.matmul(
                        ps_rep[:].rearrange("p c b -> p (c b)"),
                        lhsT=rw[64 * h:64 * (h + 1), w, :],
                        rhs=coup_t[64 * h:64 * (h + 1), :, g, :],
                        start=True, stop=True,
                    )
                    nc.vector.tensor_tensor(
                        out=cup[:, ch, :, :],
                        in0=ps_rep[:],
                        in1=up_b[:, ch, :].unsqueeze(1).broadcast_to(
                            (128, C, B)),
                        op=ALU.mult)
            # ---- (B) accumulate this half's chunks (emitted after the
            # repacks so the PE doesn't stall per chunk on the cup mults).
            # In the final half emit group1 before group0 per chunk so the
            # small group's psum closes first and its output copy overlaps
            # group0's last matmuls.
            for wl in range(4):
                w = 4 * h + wl
                for g in range(G):
                    ch = g * 8 + w
                    for (c0, nc_), ph in zip(GROUPS, bgroups):
                        nc.tensor.matmul(
                            ph[:],
                            lhsT=wp_b[:, ch, c0:c0 + nc_, :].rearrange(
                                "p c d -> p (c d)"),
                            rhs=cup[:, ch, c0:c0 + nc_, :],
                            start=(h == 0 and wl == 0 and g == 0),
                            stop=(h == 1 and wl == 3 and g == G - 1),
                            skip_group_check=True,
                        )
        groups = bgroups

        if last:
            # final round: dump the two psum groups whole (2 DMAs instead of
            # 10 tiny diagonal-block DMAs); host extracts the diagonals
            for idx, ((c0, nc_), grp) in enumerate(zip(GROUPS, groups)):
                s_f = work.tile([16 * nc_, nc_ * B], BF16, tag=f"s_f{c0}")
                nc.scalar.copy(s_f[:], grp[:])
                if idx == 0:
                    # split across two trigger queues / DMA engines
                    nc.sync.dma_start(out_d.ap()[:, 0:128], s_f[:, 0:128])
                    nc.scalar.dma_start(out_d.ap()[:, 128:256],
                                        s_f[:, 128:256])
                else:
                    nc.sync.dma_start(
                        out_d.ap()[0:16 * nc_, 256:256 + nc_ * B], s_f[:])
            return None

        def write_part(d_part):
            dp = d_part[:].rearrange("d (c b) -> d c b", c=C)
            diag_out(groups, lambda c: dp[:, c, :])
        return gather(write_part, rnd)

    stot = round0()
    squash_update(stot)
    stot = round12(1, last=False)
    squash_update(stot)
    round12(2, last=True)


IN_SHAPES = {
    "wa": [128, C * NLG * DI],
    "u4": [G * B, NLG * DI],
    "rw": [NLG, 8 * 128],
    "wp_b": [128, NCH * C * DV],
    "up_b": [128, NCH * B],
    "isid": [128, DV],
    "rep16": [DV, G * 32],
    "gmask": [128, G * B],
}


def build_nc():
    nc = bacc.Bacc("TRN2", target_bir_lowering=False, debug=False,
                   num_devices=NCORES)
    dins = {name: nc.dram_tensor(name, shape, BF16, kind="ExternalInput")
            for name, shape in IN_SHAPES.items()}
    # out: group0 (capsules 0-7) full [128,(8c,b)] dump in cols 0:256,
    # group1 (capsules 8-9) [32,(2c,b)] in cols 256:320 rows 0:32
    out_d = nc.dram_tensor("out", [128, 320], BF16, kind="ExternalOutput")

    with tile.TileContext(nc) as tc, ExitStack() as ctx:
        _body(ctx, tc, dins, out_d)
    nc.compile()
    return nc


# --------------------------------------------------------------------------
# Host side
# --------------------------------------------------------------------------

def make_in_maps(x, W):
    x = np.ascontiguousarray(np.asarray(x, dtype=np.float32))
    W = np.ascontiguousarray(np.asarray(W, dtype=np.float32))
    u = x.reshape(B, N, DI)
    isid = np.tile(np.eye(DV, dtype=np.float32), (NCORES, 1)).astype(
        ml_dtypes.bfloat16)
    gmask = np.ascontiguousarray(
        np.kron(np.eye(G, dtype=np.float32), np.ones((32, 1), np.float32))
        .reshape(128, G, 1) * np.ones((1, 1, B), np.float32)
        ).reshape(128, G * B).astype(ml_dtypes.bfloat16)
    rep16 = np.zeros((DV, G, 32), np.float32)
    for d in range(DV):
        rep16[d, :, d] = 1.0
    rep16 = rep16.reshape(DV, G * 32).astype(ml_dtypes.bfloat16)
    rwm = np.zeros((NLG, 8, 128), np.float32)
    for w in range(8):
        for n16 in range(16):
            rwm[w * 16 + n16, w, n16 * DI:(n16 + 1) * DI] = 1.0
    rwm = rwm.reshape(NLG, 8 * 128).astype(ml_dtypes.bfloat16)

    in_maps = []
    for k in range(NCORES):
        sl = u[:, k * NL:(k + 1) * NL, :]                   # [B, 512, 8]
        Wk = W[k * NL:(k + 1) * NL]                         # [512, C, DI, DV]
        Wk_g = Wk.reshape(G, NLG, C, DI, DV)
        wa = np.zeros((G, 32, C, NLG, DI), np.float32)
        wa[:, :DV] = Wk_g.transpose(0, 4, 2, 1, 3)          # [g,d,c,nl,i]
        # packed: p = (n16, i), chunks of 16 n
        Wp = Wk.reshape(NCH, 16, C, DI, DV).transpose(1, 3, 0, 2, 4)
        # -> [n16, i, ch, c, d]
        Up = sl.reshape(B, NCH, 16, DI).transpose(2, 3, 1, 0)  # [n16,i,ch,b]
        u4 = sl.reshape(B, G, NLG, DI).transpose(1, 0, 2, 3)   # [g,b,nl,i]
        in_maps.append({
            "wa": np.ascontiguousarray(
                wa.reshape(128, C * NLG * DI)).astype(ml_dtypes.bfloat16),
            "wp_b": np.ascontiguousarray(
                Wp.reshape(128, NCH * C * DV)).astype(ml_dtypes.bfloat16),
            "up_b": np.ascontiguousarray(
                Up.reshape(128, NCH * B)).astype(ml_dtypes.bfloat16),
            "u4": np.ascontiguousarray(
                u4.reshape(G * B, NLG * DI)).astype(ml_dtypes.bfloat16),
            "rw": rwm,
            "isid": isid,
            "rep16": rep16,
            "gmask": gmask,
        })
    return in_maps


def postprocess(outs):
    """outs: list (per core) of [128, 320] bf16 full psum-group dumps.
    Diagonal blocks: group0 s[c,d,b] = o[c*16+d, c*32+b] (c<8), group1
    s[8+c,d,b] = o[c*16+d, 256+c*32+b] (c<2). Sum cores, squash."""
    s = np.zeros((C, DV, B), np.float64)
    i8, i2 = np.arange(8), np.arange(2)
    for o in outs:
        o = np.asarray(o, np.float64)
        s[:8] += o[:, :256].reshape(8, DV, 8, B)[i8, :, i8, :]
        s[8:] += o[:32, 256:].reshape(2, DV, 2, B)[i2, :, i2, :]
    s = s.transpose(2, 0, 1)                                # [b, c, d]
    s2 = np.sum(s * s, axis=-1, keepdims=True)
    v = (s2 / (1.0 + s2) / np.sqrt(s2 + EPS)) * s
    return v.astype(np.float32)


_NC_CACHE = {}


def kernel(x, W):
    if "nc" not in _NC_CACHE:
        _NC_CACHE["nc"] = build_nc()
    nc = _NC_CACHE["nc"]
    in_maps = make_in_maps(x, W)
    res = run_bass_kernel_spmd(nc, in_maps, list(range(NCORES)))
    outs = [res.results[k]["out"] for k in range(NCORES)]
    return postprocess(outs)
